# revision 9
# baseline (speedup 1.0000x reference)
"""3-layer GCN (AdjGCN) on 8 Trainium2 NeuronCores.

Strategy (aggregate-first): out_l = relu((A @ x) @ W_l + b_l), so the dense
matmul is local to the dst shard. Nodes (dst rows) are block-sharded across
the 8 cores; each core processes the in-edges of its 12500 rows.

Per layer, per core:
  - gather x[src] rows from HBM via 4-queue SWDGE dma_gather (int16 indices
    force a 4-way src-chunk split of the table),
  - weighted segment-sum via PE matmuls against host-built one-hot blocks
    S_w[e, m] = w_e * (dst_local(e) == m), accumulated in PSUM feature-major
    per 128-row dst window,
  - layer matmul + bias + relu per window,
  - AllGather of the shard into the next layer's gather table.

The bass program is compiled per-call with the graph baked in (edge->tile
assignment is static; per-core differences live in the input arrays, whose
shapes are uniform across cores).
"""
import math
import os

import numpy as np
import ml_dtypes

import concourse.bass as bass
import concourse.bacc as bacc
import concourse.mybir as mybir
from concourse.tile import TileContext
from concourse.bass_utils import run_bass_kernel_spmd
from concourse.library_config import mlp

P = 128
N = 100000
E = 1600000
D_IN = 128
D_HID = 128
D_OUT = 40
R = 8                      # cores
S = N // R                 # dst rows per core
NW = math.ceil(S / P)      # dst windows per core (last is partial)
NCHUNK = 4                 # src chunks (int16 index limit is 32768 rows)
CH = N // NCHUNK
SUPER = 6                  # windows per super-group (6 seg psums + 2 psum2 = 8 banks)
CALL_TILES = 16            # max 128-row tiles per dma_gather call (2048 idx)

# Toggles (module-level so test.py can flip them)
TRACE = False
LAST_EXEC_NS = None
USE_BF16 = True

_BF16 = ml_dtypes.bfloat16


def _prep_graph(edge_index, edge_weight, np_dt):
    """Sort/partition edges; build per-core idx + S_w arrays and the static
    call/tile schedule (uniform across cores)."""
    src = np.asarray(edge_index[0], dtype=np.int64)
    dst = np.asarray(edge_index[1], dtype=np.int64)
    w = np.asarray(edge_weight, dtype=np.float32)

    core = dst // S
    wloc = (dst % S) // P
    mloc = (dst % S) % P
    chunk = src // CH
    cidx = (src % CH).astype(np.int16)

    order = np.lexsort((src, chunk, wloc, core))
    src, dst, w, core, wloc, mloc, chunk, cidx = (
        a[order] for a in (src, dst, w, core, wloc, mloc, chunk, cidx))

    # counts[core, w, c]
    key = (core * NW + wloc) * NCHUNK + chunk
    counts = np.bincount(key, minlength=R * NW * NCHUNK).reshape(R, NW, NCHUNK)
    tiles_wc = np.ceil(counts.max(axis=0) / P).astype(np.int64)  # [NW, NCHUNK]
    # make sure every (w,c) with any edges anywhere has >=1 tile; zero-tile ok
    # global tile base per (w, c) in (w-major, c-minor? no: schedule order) --
    # tile ids are assigned in schedule order: super -> chunk -> window -> tile
    tile_base = np.zeros((NW, NCHUNK), dtype=np.int64)
    tid = 0
    schedule = []  # list of supers; each: list of calls; call = (c, [(w, gtile, tt)])
    nsup = math.ceil(NW / SUPER)
    for sg in range(nsup):
        ws = list(range(sg * SUPER, min((sg + 1) * SUPER, NW)))
        sup_calls = []
        for c in range(NCHUNK):
            pend = []  # tiles (w, gtile) for this (super, c)
            for wq in ws:
                tile_base[wq, c] = tid
                for _ in range(int(tiles_wc[wq, c])):
                    pend.append((wq, tid))
                    tid += 1
            for i0 in range(0, len(pend), CALL_TILES):
                grp = pend[i0:i0 + CALL_TILES]
                sup_calls.append((c, [(wq, gt, j) for j, (wq, gt) in enumerate(grp)]))
        schedule.append((ws, sup_calls))
    NT = tid

    # first/last tile per window (for PSUM start/stop flags)
    first_t = np.full(NW, -1, dtype=np.int64)
    last_t = np.full(NW, -1, dtype=np.int64)
    for wq in range(NW):
        for c in range(NCHUNK):
            t0, nt = tile_base[wq, c], tiles_wc[wq, c]
            if nt > 0:
                if first_t[wq] < 0:
                    first_t[wq] = t0
                last_t[wq] = t0 + nt - 1

    # per-edge destination tile/slot (per core)
    # position within (core, w, c) group:
    grp_key = key
    # stable sort already groups; compute within-group positions
    uniq, inv, cnt = np.unique(grp_key, return_inverse=True, return_counts=True)
    starts = np.zeros_like(cnt)
    np.cumsum(cnt[:-1], out=starts[1:])
    pos = np.arange(len(grp_key)) - starts[inv]
    gtile = tile_base[wloc, chunk] + pos // P
    slot = pos % P

    idx_tiles = [np.zeros((NT, P), dtype=np.int16) for _ in range(R)]
    sw_tiles = [np.zeros((NT, P, P), dtype=np_dt) for _ in range(R)]
    for r in range(R):
        m = core == r
        idx_tiles[r][gtile[m], slot[m]] = cidx[m]
        sw_tiles[r][gtile[m], slot[m], mloc[m]] = w[m].astype(np_dt)

    # flatten per-call blocks
    call_meta = []  # (c, idx_off, idx_w, sw_off, tiles_meta, num_idxs)
    idx_blocks = [[] for _ in range(R)]
    sw_blocks = [[] for _ in range(R)]
    idx_off = 0
    sw_off = 0
    for ws, sup_calls in schedule:
        for c, tl in sup_calls:
            tc = len(tl)
            ni = tc * P
            t0 = tl[0][1]
            for r in range(R):
                arr = idx_tiles[r][t0:t0 + tc].reshape(tc * P)
                wrapped = arr.reshape(tc * 8, 16).T  # [16, 8*tc]
                idx_blocks[r].append(np.tile(wrapped, (8, 1)).ravel())
                swb = sw_tiles[r][t0:t0 + tc].transpose(1, 0, 2).reshape(P, tc * P)
                sw_blocks[r].append(np.ascontiguousarray(swb).ravel())
            call_meta.append((c, idx_off, 8 * tc, sw_off, tl, ni))
            idx_off += P * 8 * tc
            sw_off += P * tc * P
    idx_flat = [np.concatenate(b) for b in idx_blocks]
    sw_flat = [np.concatenate(b) for b in sw_blocks]

    return dict(schedule=schedule, call_meta=call_meta, first_t=first_t,
                last_t=last_t, idx_flat=idx_flat, sw_flat=sw_flat,
                idx_total=idx_off, sw_total=sw_off, NT=NT)


def _build_program(g, dt, np_dt):
    nc = bacc.Bacc("TRN2", target_bir_lowering=False, num_swdge_queues=4,
                   dynamic_dma_scratch_size=int(os.environ.get("GCN_DDS", 16384)))
    f32 = mybir.dt.float32
    AF = mybir.ActivationFunctionType
    ALU = mybir.AluOpType

    x0 = nc.dram_tensor("x0", [N, P], dt, kind="ExternalInput")
    idx_in = nc.dram_tensor("idxf", [g["idx_total"]], mybir.dt.int16,
                            kind="ExternalInput")
    sw_in = nc.dram_tensor("swf", [g["sw_total"]], dt, kind="ExternalInput")
    w1_in = nc.dram_tensor("w1", [P, D_HID], dt, kind="ExternalInput")
    w2_in = nc.dram_tensor("w2", [P, D_HID], dt, kind="ExternalInput")
    w3_in = nc.dram_tensor("w3", [P, D_OUT], f32, kind="ExternalInput")
    b1_in = nc.dram_tensor("b1", [1, D_HID], dt, kind="ExternalInput")
    b2_in = nc.dram_tensor("b2", [1, D_HID], dt, kind="ExternalInput")
    b3_in = nc.dram_tensor("b3", [1, D_OUT], f32, kind="ExternalInput")
    ones_in = nc.dram_tensor("ones", [1, P], dt, kind="ExternalInput")
    ones32_in = nc.dram_tensor("ones32", [1, P], f32, kind="ExternalInput")
    out = nc.dram_tensor("out", [S, D_OUT], f32, kind="ExternalOutput")

    xshard = nc.dram_tensor("xshard", [S, P], dt)
    xbuf = nc.dram_tensor("xbuf", [N, P], dt)

    call_meta = g["call_meta"]
    schedule = g["schedule"]
    first_t = g["first_t"]
    last_t = g["last_t"]

    with TileContext(nc) as tc:
        with (
            tc.tile_pool(name="const", bufs=1) as cp,
            tc.tile_pool(name="idx", bufs=24) as ip,
            tc.tile_pool(name="sw", bufs=12) as swp,
            tc.tile_pool(name="g", bufs=12) as gpool,
            tc.tile_pool(name="segp", bufs=SUPER, space="PSUM") as segp,
            tc.tile_pool(name="mmp", bufs=2, space="PSUM") as mmp,
            tc.tile_pool(name="work", bufs=6) as wk,
        ):
            nc.gpsimd.load_library(mlp)
            w1 = cp.tile([P, D_HID], dt, tag="w1")
            w2 = cp.tile([P, D_HID], dt, tag="w2")
            w3 = cp.tile([P, D_OUT], f32, tag="w3")
            b1 = cp.tile([1, D_HID], dt, tag="b1")
            b2 = cp.tile([1, D_HID], dt, tag="b2")
            b3 = cp.tile([1, D_OUT], f32, tag="b3")
            ones = cp.tile([1, P], dt, tag="ones")
            ones32 = cp.tile([1, P], f32, tag="ones32")
            for t, src_t in ((w1, w1_in), (w2, w2_in), (w3, w3_in), (b1, b1_in),
                             (b2, b2_in), (b3, b3_in), (ones, ones_in),
                             (ones32, ones32_in)):
                nc.sync.dma_start(out=t[:], in_=src_t[:])

            for layer in range(3):
                table = x0 if layer == 0 else xbuf
                wl = (w1, w2, w3)[layer]
                bl = (b1, b2, b3)[layer]
                d_out_l = (D_HID, D_HID, D_OUT)[layer]
                ci = 0
                for ws, sup_calls in schedule:
                    psums = {}
                    for wq in ws:
                        psums[wq] = segp.tile([P, P], f32, tag="seg", name=f"seg{wq}")
                    idx_tiles_sb = []
                    for k, (c, tl) in enumerate(sup_calls):
                        _, ioff, iw, _, _, _ = call_meta[ci + k]
                        idx_sb = ip.tile([P, 8 * CALL_TILES], mybir.dt.int16,
                                         tag="idx", name=f"idx{k}")
                        nc.sync.dma_start(
                            out=idx_sb[:, :iw],
                            in_=idx_in[ioff:ioff + P * iw].rearrange(
                                "(p c) -> p c", p=P))
                        idx_tiles_sb.append(idx_sb)
                    for k, (c, tl) in enumerate(sup_calls):
                        c_, ioff, iw, soff, _, ni = call_meta[ci]
                        assert c_ == c
                        tcnt = len(tl)
                        idx_sb = idx_tiles_sb[k]
                        sw_sb = swp.tile([P, CALL_TILES * P], dt, tag="sw")
                        nc.scalar.dma_start(
                            out=sw_sb[:, :tcnt * P],
                            in_=sw_in[soff:soff + P * tcnt * P].rearrange(
                                "(p c) -> p c", p=P))
                        gt = gpool.tile([P, CALL_TILES * P], dt, tag="g")
                        nc.gpsimd.dma_gather(
                            out_ap=gt[:, :tcnt * P].rearrange(
                                "p (t f) -> p t f", t=tcnt),
                            in_ap=table[c * CH:(c + 1) * CH, :],
                            idxs_ap=idx_sb[:, :iw],
                            num_idxs=ni,
                            num_idxs_reg=ni,
                            elem_size=P,
                            single_packet=False,
                            queue_num=ci % 4,
                        )
                        for wq, gtile, tt in tl:
                            nc.tensor.matmul(
                                psums[wq][:],
                                lhsT=gt[:, tt * P:(tt + 1) * P],
                                rhs=sw_sb[:, tt * P:(tt + 1) * P],
                                start=bool(gtile == first_t[wq]),
                                stop=bool(gtile == last_t[wq]),
                            )
                        ci += 1
                    # drain windows of this super-group
                    for wq in ws:
                        nw = min(P, S - wq * P)
                        if layer < 2:
                            aggT = wk.tile([P, P], dt, tag="aggT")
                            nc.scalar.copy(out=aggT[:], in_=psums[wq][:])
                            ps2 = mmp.tile([P, D_HID], f32, tag="mm")
                            nc.tensor.matmul(ps2[:nw, :], lhsT=ones[:, :nw],
                                             rhs=bl[:], start=True, stop=False)
                            nc.tensor.matmul(ps2[:nw, :], lhsT=aggT[:, :nw],
                                             rhs=wl[:], start=False, stop=True)
                            xn = wk.tile([P, P], dt, tag="xn")
                            nc.scalar.activation(out=xn[:nw, :], in_=ps2[:nw, :],
                                                 func=AF.Relu)
                            nc.scalar.dma_start(
                                out=xshard[wq * P:wq * P + nw, :],
                                in_=xn[:nw, :])
                        else:
                            aggT = wk.tile([P, P], f32, tag="aggT32")
                            nc.scalar.copy(out=aggT[:], in_=psums[wq][:])
                            ps2_full = mmp.tile([P, D_HID], f32, tag="mm", name="ps2f")
                            ps2 = ps2_full[:, :D_OUT]
                            nc.tensor.matmul(ps2[:nw, :], lhsT=ones32[:, :nw],
                                             rhs=bl[:], start=True, stop=False)
                            nc.tensor.matmul(ps2[:nw, :], lhsT=aggT[:, :nw],
                                             rhs=wl[:], start=False, stop=True)
                            nm1 = wk.tile([P, 1], f32, tag="nm1")
                            nc.vector.reduce_max(nm1[:nw, :], ps2[:nw, :],
                                                 axis=mybir.AxisListType.X,
                                                 negate=True)
                            esb = wk.tile([P, D_OUT], f32, tag="esb")
                            nc.scalar.activation(out=esb[:nw, :], in_=ps2[:nw, :],
                                                 func=AF.Exp, bias=nm1[:nw, :])
                            ssum = wk.tile([P, 1], f32, tag="ssum")
                            nc.vector.reduce_sum(ssum[:nw, :], esb[:nw, :],
                                                 axis=mybir.AxisListType.X)
                            lse = wk.tile([P, 1], f32, tag="lse")
                            nc.scalar.activation(out=lse[:nw, :], in_=ssum[:nw, :],
                                                 func=AF.Ln)
                            osb = wk.tile([P, D_OUT], f32, tag="osb")
                            nc.vector.tensor_scalar(
                                out=osb[:nw, :], in0=ps2[:nw, :],
                                scalar1=nm1[:nw, :], scalar2=lse[:nw, :],
                                op0=ALU.add, op1=ALU.subtract)
                            nc.scalar.dma_start(
                                out=out[wq * P:wq * P + nw, :],
                                in_=osb[:nw, :])
                if layer < 2:
                    nc.gpsimd.collective_compute(
                        "AllGather",
                        mybir.AluOpType.bypass,
                        replica_groups=[list(range(R))],
                        ins=[xshard[:, :]],
                        outs=[xbuf[:, :]],
                    )
                    if os.environ.get("GCN_BARRIER", "1") == "1":
                        tc.strict_bb_all_engine_barrier()
    nc.finalize()
    return nc


def kernel(x, edge_index, edge_weight, W1, b1, W2, b2, W3, b3):
    global LAST_EXEC_NS
    np_dt = _BF16 if USE_BF16 else np.float32
    dt = mybir.dt.bfloat16 if USE_BF16 else mybir.dt.float32

    g = _prep_graph(edge_index, edge_weight, np_dt)
    nc = _build_program(g, dt, np_dt)

    x_h = np.ascontiguousarray(np.asarray(x, dtype=np.float32).astype(np_dt))
    w1_h = np.asarray(W1, dtype=np.float32).astype(np_dt)
    w2_h = np.asarray(W2, dtype=np.float32).astype(np_dt)
    w3_h = np.asarray(W3, dtype=np.float32)
    b1_h = np.asarray(b1, dtype=np.float32).astype(np_dt).reshape(1, D_HID)
    b2_h = np.asarray(b2, dtype=np.float32).astype(np_dt).reshape(1, D_HID)
    b3_h = np.asarray(b3, dtype=np.float32).reshape(1, D_OUT)
    ones_h = np.ones((1, P), dtype=np.float32).astype(np_dt)
    ones32_h = np.ones((1, P), dtype=np.float32)

    in_maps = []
    for r in range(R):
        in_maps.append({
            "x0": x_h, "idxf": g["idx_flat"][r], "swf": g["sw_flat"][r],
            "w1": w1_h, "w2": w2_h, "w3": w3_h,
            "b1": b1_h, "b2": b2_h, "b3": b3_h,
            "ones": ones_h, "ones32": ones32_h,
        })

    res = run_bass_kernel_spmd(nc, in_maps, core_ids=list(range(R)),
                               trace=TRACE)
    LAST_EXEC_NS = res.exec_time_ns
    if TRACE and res.instructions_and_trace is not None and os.environ.get("GCN_DUMP"):
        import pickle
        insts, _tp = res.instructions_and_trace
        rows = [(i.name, i.engine, i.timestamp, i.duration, i.evt_wait_time,
                 str(i.critical_dep), i.label, i.op_name) for i in insts]
        with open(os.environ["GCN_DUMP"], "wb") as fh:
            pickle.dump(rows, fh)
    return np.concatenate([res.results[r]["out"] for r in range(R)], axis=0)


# revision 10
# speedup vs baseline: 1.1541x; 1.1541x over previous
"""3-layer GCN (AdjGCN) on 8 Trainium2 NeuronCores.

Strategy (aggregate-first): out_l = relu((A @ x) @ W_l + b_l), so the dense
matmul is local to the dst shard. Nodes (dst rows) are block-sharded across
the 8 cores; each core processes the in-edges of its 12500 rows.

Per layer, per core:
  - gather x[src] rows from HBM via 4-queue SWDGE dma_gather (int16 indices
    force a 4-way src-chunk split of the table),
  - weighted segment-sum via PE matmuls against host-built one-hot blocks
    S_w[e, m] = w_e * (dst_local(e) == m), accumulated in PSUM feature-major
    per 128-row dst window,
  - layer matmul + bias + relu per window,
  - AllGather of the shard into the next layer's gather table.

The bass program is compiled per-call with the graph baked in (edge->tile
assignment is static; per-core differences live in the input arrays, whose
shapes are uniform across cores).
"""
import math
import os

import numpy as np
import ml_dtypes

import concourse.bass as bass
import concourse.bacc as bacc
import concourse.mybir as mybir
from concourse.tile import TileContext
from concourse.bass_utils import run_bass_kernel_spmd
from concourse.library_config import mlp

P = 128
N = 100000
E = 1600000
D_IN = 128
D_HID = 128
D_OUT = 40
R = 8                      # cores
S = N // R                 # dst rows per core
NW = math.ceil(S / P)      # dst windows per core (last is partial)
NCHUNK = 4                 # src chunks (int16 index limit is 32768 rows)
CH = N // NCHUNK
SUPER = 6                  # windows per super-group (6 seg psums + 2 psum2 = 8 banks)
CALL_TILES = 16            # max 128-row tiles per dma_gather call (2048 idx)

# Toggles (module-level so test.py can flip them)
TRACE = False
LAST_EXEC_NS = None
USE_BF16 = True

_BF16 = ml_dtypes.bfloat16


def _prep_graph(edge_index, edge_weight, np_dt):
    """Sort/partition edges; build per-core idx + S_w arrays and the static
    call/tile schedule (uniform across cores)."""
    src = np.asarray(edge_index[0], dtype=np.int64)
    dst = np.asarray(edge_index[1], dtype=np.int64)
    w = np.asarray(edge_weight, dtype=np.float32)

    core = dst // S
    wloc = (dst % S) // P
    mloc = (dst % S) % P
    chunk = src // CH
    cidx = (src % CH).astype(np.int16)

    order = np.lexsort((src, chunk, wloc, core))
    src, dst, w, core, wloc, mloc, chunk, cidx = (
        a[order] for a in (src, dst, w, core, wloc, mloc, chunk, cidx))

    # counts[core, w, c]
    key = (core * NW + wloc) * NCHUNK + chunk
    counts = np.bincount(key, minlength=R * NW * NCHUNK).reshape(R, NW, NCHUNK)
    tiles_wc = np.ceil(counts.max(axis=0) / P).astype(np.int64)  # [NW, NCHUNK]
    # make sure every (w,c) with any edges anywhere has >=1 tile; zero-tile ok
    # global tile base per (w, c) in (w-major, c-minor? no: schedule order) --
    # tile ids are assigned in schedule order: super -> chunk -> window -> tile
    tile_base = np.zeros((NW, NCHUNK), dtype=np.int64)
    tid = 0
    schedule = []  # list of supers; each: list of calls; call = (c, [(w, gtile, tt)])
    nsup = math.ceil(NW / SUPER)
    for sg in range(nsup):
        ws = list(range(sg * SUPER, min((sg + 1) * SUPER, NW)))
        sup_calls = []
        for c in range(NCHUNK):
            pend = []  # tiles (w, gtile) for this (super, c)
            for wq in ws:
                tile_base[wq, c] = tid
                for _ in range(int(tiles_wc[wq, c])):
                    pend.append((wq, tid))
                    tid += 1
            for i0 in range(0, len(pend), CALL_TILES):
                grp = pend[i0:i0 + CALL_TILES]
                sup_calls.append((c, [(wq, gt, j) for j, (wq, gt) in enumerate(grp)]))
        schedule.append((ws, sup_calls))
    NT = tid

    # first/last tile per window (for PSUM start/stop flags)
    first_t = np.full(NW, -1, dtype=np.int64)
    last_t = np.full(NW, -1, dtype=np.int64)
    for wq in range(NW):
        for c in range(NCHUNK):
            t0, nt = tile_base[wq, c], tiles_wc[wq, c]
            if nt > 0:
                if first_t[wq] < 0:
                    first_t[wq] = t0
                last_t[wq] = t0 + nt - 1

    # per-edge destination tile/slot (per core)
    # position within (core, w, c) group:
    grp_key = key
    # stable sort already groups; compute within-group positions
    uniq, inv, cnt = np.unique(grp_key, return_inverse=True, return_counts=True)
    starts = np.zeros_like(cnt)
    np.cumsum(cnt[:-1], out=starts[1:])
    pos = np.arange(len(grp_key)) - starts[inv]
    gtile = tile_base[wloc, chunk] + pos // P
    slot = pos % P

    idx_tiles = [np.zeros((NT, P), dtype=np.int16) for _ in range(R)]
    sw_tiles = [np.zeros((NT, P, P), dtype=np_dt) for _ in range(R)]
    for r in range(R):
        m = core == r
        idx_tiles[r][gtile[m], slot[m]] = cidx[m]
        sw_tiles[r][gtile[m], slot[m], mloc[m]] = w[m].astype(np_dt)

    # flatten per-call blocks
    call_meta = []  # (c, idx_off, idx_w, sw_off, tiles_meta, num_idxs)
    idx_blocks = [[] for _ in range(R)]
    sw_blocks = [[] for _ in range(R)]
    idx_off = 0
    sw_off = 0
    for ws, sup_calls in schedule:
        for c, tl in sup_calls:
            tc = len(tl)
            ni = tc * P
            t0 = tl[0][1]
            for r in range(R):
                arr = idx_tiles[r][t0:t0 + tc].reshape(tc * P)
                wrapped = arr.reshape(tc * 8, 16).T  # [16, 8*tc]
                idx_blocks[r].append(np.tile(wrapped, (8, 1)).ravel())
                swb = sw_tiles[r][t0:t0 + tc].transpose(1, 0, 2).reshape(P, tc * P)
                sw_blocks[r].append(np.ascontiguousarray(swb).ravel())
            call_meta.append((c, idx_off, 8 * tc, sw_off, tl, ni))
            idx_off += P * 8 * tc
            sw_off += P * tc * P
    idx_flat = [np.concatenate(b) for b in idx_blocks]
    sw_flat = [np.concatenate(b) for b in sw_blocks]

    return dict(schedule=schedule, call_meta=call_meta, first_t=first_t,
                last_t=last_t, idx_flat=idx_flat, sw_flat=sw_flat,
                idx_total=idx_off, sw_total=sw_off, NT=NT)


def _build_program(g, dt, np_dt):
    nc = bacc.Bacc("TRN2", target_bir_lowering=False, num_swdge_queues=4,
                   dynamic_dma_scratch_size=int(os.environ.get("GCN_DDS", 16384)))
    f32 = mybir.dt.float32
    AF = mybir.ActivationFunctionType
    ALU = mybir.AluOpType

    x0 = nc.dram_tensor("x0", [N, P], dt, kind="ExternalInput")
    idx_in = nc.dram_tensor("idxf", [g["idx_total"]], mybir.dt.int16,
                            kind="ExternalInput")
    sw_in = nc.dram_tensor("swf", [g["sw_total"]], dt, kind="ExternalInput")
    w1_in = nc.dram_tensor("w1", [P, D_HID], dt, kind="ExternalInput")
    w2_in = nc.dram_tensor("w2", [P, D_HID], dt, kind="ExternalInput")
    w3_in = nc.dram_tensor("w3", [P, D_OUT], f32, kind="ExternalInput")
    b1_in = nc.dram_tensor("b1", [1, D_HID], dt, kind="ExternalInput")
    b2_in = nc.dram_tensor("b2", [1, D_HID], dt, kind="ExternalInput")
    b3_in = nc.dram_tensor("b3", [1, D_OUT], f32, kind="ExternalInput")
    ones_in = nc.dram_tensor("ones", [1, P], dt, kind="ExternalInput")
    ones32_in = nc.dram_tensor("ones32", [1, P], f32, kind="ExternalInput")
    out = nc.dram_tensor("out", [S, D_OUT], f32, kind="ExternalOutput")

    xshard = nc.dram_tensor("xshard", [S, P], dt)
    xbuf = nc.dram_tensor("xbuf", [N, P], dt)

    call_meta = g["call_meta"]
    schedule = g["schedule"]
    first_t = g["first_t"]
    last_t = g["last_t"]

    with TileContext(nc) as tc:
        with (
            tc.tile_pool(name="const", bufs=1) as cp,
            tc.tile_pool(name="idx", bufs=24) as ip,
            tc.tile_pool(name="sw", bufs=12) as swp,
            tc.tile_pool(name="g", bufs=12) as gpool,
            tc.tile_pool(name="segp", bufs=SUPER, space="PSUM") as segp,
            tc.tile_pool(name="mmp", bufs=2, space="PSUM") as mmp,
            tc.tile_pool(name="work", bufs=6) as wk,
        ):
            nc.gpsimd.load_library(mlp)
            w1 = cp.tile([P, D_HID], dt, tag="w1")
            w2 = cp.tile([P, D_HID], dt, tag="w2")
            w3 = cp.tile([P, D_OUT], f32, tag="w3")
            b1 = cp.tile([1, D_HID], dt, tag="b1")
            b2 = cp.tile([1, D_HID], dt, tag="b2")
            b3 = cp.tile([1, D_OUT], f32, tag="b3")
            ones = cp.tile([1, P], dt, tag="ones")
            ones32 = cp.tile([1, P], f32, tag="ones32")
            for t, src_t in ((w1, w1_in), (w2, w2_in), (w3, w3_in), (b1, b1_in),
                             (b2, b2_in), (b3, b3_in), (ones, ones_in),
                             (ones32, ones32_in)):
                nc.sync.dma_start(out=t[:], in_=src_t[:])

            for layer in range(3):
                table = x0 if layer == 0 else xbuf
                wl = (w1, w2, w3)[layer]
                bl = (b1, b2, b3)[layer]
                d_out_l = (D_HID, D_HID, D_OUT)[layer]
                ci = 0
                for ws, sup_calls in schedule:
                    psums = {}
                    for wq in ws:
                        psums[wq] = segp.tile([P, P], f32, tag="seg", name=f"seg{wq}")
                    idx_tiles_sb = []
                    for k, (c, tl) in enumerate(sup_calls):
                        _, ioff, iw, _, _, _ = call_meta[ci + k]
                        idx_sb = ip.tile([P, 8 * CALL_TILES], mybir.dt.int16,
                                         tag="idx", name=f"idx{k}")
                        nc.scalar.dma_start(
                            out=idx_sb[:, :iw],
                            in_=idx_in[ioff:ioff + P * iw].rearrange(
                                "(p c) -> p c", p=P))
                        idx_tiles_sb.append(idx_sb)
                    for k, (c, tl) in enumerate(sup_calls):
                        c_, ioff, iw, soff, _, ni = call_meta[ci]
                        assert c_ == c
                        tcnt = len(tl)
                        idx_sb = idx_tiles_sb[k]
                        sw_sb = swp.tile([P, CALL_TILES * P], dt, tag="sw")
                        nc.sync.dma_start(
                            out=sw_sb[:, :tcnt * P],
                            in_=sw_in[soff:soff + P * tcnt * P].rearrange(
                                "(p c) -> p c", p=P))
                        gt = gpool.tile([P, CALL_TILES * P], dt, tag="g")
                        nc.gpsimd.dma_gather(
                            out_ap=gt[:, :tcnt * P].rearrange(
                                "p (t f) -> p t f", t=tcnt),
                            in_ap=table[c * CH:(c + 1) * CH, :],
                            idxs_ap=idx_sb[:, :iw],
                            num_idxs=ni,
                            num_idxs_reg=ni,
                            elem_size=P,
                            single_packet=False,
                            queue_num=ci % 4,
                        )
                        for wq, gtile, tt in tl:
                            nc.tensor.matmul(
                                psums[wq][:],
                                lhsT=gt[:, tt * P:(tt + 1) * P],
                                rhs=sw_sb[:, tt * P:(tt + 1) * P],
                                start=bool(gtile == first_t[wq]),
                                stop=bool(gtile == last_t[wq]),
                            )
                        ci += 1
                    # drain windows of this super-group
                    for wq in ws:
                        nw = min(P, S - wq * P)
                        if layer < 2:
                            aggT = wk.tile([P, P], dt, tag="aggT")
                            nc.scalar.copy(out=aggT[:], in_=psums[wq][:])
                            ps2 = mmp.tile([P, D_HID], f32, tag="mm")
                            nc.tensor.matmul(ps2[:nw, :], lhsT=ones[:, :nw],
                                             rhs=bl[:], start=True, stop=False)
                            nc.tensor.matmul(ps2[:nw, :], lhsT=aggT[:, :nw],
                                             rhs=wl[:], start=False, stop=True)
                            xn = wk.tile([P, P], dt, tag="xn")
                            nc.scalar.activation(out=xn[:nw, :], in_=ps2[:nw, :],
                                                 func=AF.Relu)
                            nc.scalar.dma_start(
                                out=xshard[wq * P:wq * P + nw, :],
                                in_=xn[:nw, :])
                        else:
                            aggT = wk.tile([P, P], f32, tag="aggT32")
                            nc.scalar.copy(out=aggT[:], in_=psums[wq][:])
                            ps2_full = mmp.tile([P, D_HID], f32, tag="mm", name="ps2f")
                            ps2 = ps2_full[:, :D_OUT]
                            nc.tensor.matmul(ps2[:nw, :], lhsT=ones32[:, :nw],
                                             rhs=bl[:], start=True, stop=False)
                            nc.tensor.matmul(ps2[:nw, :], lhsT=aggT[:, :nw],
                                             rhs=wl[:], start=False, stop=True)
                            nm1 = wk.tile([P, 1], f32, tag="nm1")
                            nc.vector.reduce_max(nm1[:nw, :], ps2[:nw, :],
                                                 axis=mybir.AxisListType.X,
                                                 negate=True)
                            esb = wk.tile([P, D_OUT], f32, tag="esb")
                            nc.scalar.activation(out=esb[:nw, :], in_=ps2[:nw, :],
                                                 func=AF.Exp, bias=nm1[:nw, :])
                            ssum = wk.tile([P, 1], f32, tag="ssum")
                            nc.vector.reduce_sum(ssum[:nw, :], esb[:nw, :],
                                                 axis=mybir.AxisListType.X)
                            lse = wk.tile([P, 1], f32, tag="lse")
                            nc.scalar.activation(out=lse[:nw, :], in_=ssum[:nw, :],
                                                 func=AF.Ln)
                            osb = wk.tile([P, D_OUT], f32, tag="osb")
                            nc.vector.tensor_scalar(
                                out=osb[:nw, :], in0=ps2[:nw, :],
                                scalar1=nm1[:nw, :], scalar2=lse[:nw, :],
                                op0=ALU.add, op1=ALU.subtract)
                            nc.scalar.dma_start(
                                out=out[wq * P:wq * P + nw, :],
                                in_=osb[:nw, :])
                if layer < 2:
                    nc.gpsimd.collective_compute(
                        "AllGather",
                        mybir.AluOpType.bypass,
                        replica_groups=[list(range(R))],
                        ins=[xshard[:, :]],
                        outs=[xbuf[:, :]],
                    )
                    if os.environ.get("GCN_BARRIER", "1") == "1":
                        tc.strict_bb_all_engine_barrier()
    nc.finalize()
    return nc


def kernel(x, edge_index, edge_weight, W1, b1, W2, b2, W3, b3):
    global LAST_EXEC_NS
    np_dt = _BF16 if USE_BF16 else np.float32
    dt = mybir.dt.bfloat16 if USE_BF16 else mybir.dt.float32

    g = _prep_graph(edge_index, edge_weight, np_dt)
    nc = _build_program(g, dt, np_dt)

    x_h = np.ascontiguousarray(np.asarray(x, dtype=np.float32).astype(np_dt))
    w1_h = np.asarray(W1, dtype=np.float32).astype(np_dt)
    w2_h = np.asarray(W2, dtype=np.float32).astype(np_dt)
    w3_h = np.asarray(W3, dtype=np.float32)
    b1_h = np.asarray(b1, dtype=np.float32).astype(np_dt).reshape(1, D_HID)
    b2_h = np.asarray(b2, dtype=np.float32).astype(np_dt).reshape(1, D_HID)
    b3_h = np.asarray(b3, dtype=np.float32).reshape(1, D_OUT)
    ones_h = np.ones((1, P), dtype=np.float32).astype(np_dt)
    ones32_h = np.ones((1, P), dtype=np.float32)

    in_maps = []
    for r in range(R):
        in_maps.append({
            "x0": x_h, "idxf": g["idx_flat"][r], "swf": g["sw_flat"][r],
            "w1": w1_h, "w2": w2_h, "w3": w3_h,
            "b1": b1_h, "b2": b2_h, "b3": b3_h,
            "ones": ones_h, "ones32": ones32_h,
        })

    res = run_bass_kernel_spmd(nc, in_maps, core_ids=list(range(R)),
                               trace=TRACE)
    LAST_EXEC_NS = res.exec_time_ns
    if TRACE and res.instructions_and_trace is not None and os.environ.get("GCN_DUMP"):
        import pickle
        insts, _tp = res.instructions_and_trace
        rows = [(i.name, i.engine, i.timestamp, i.duration, i.evt_wait_time,
                 str(i.critical_dep), i.label, i.op_name) for i in insts]
        with open(os.environ["GCN_DUMP"], "wb") as fh:
            pickle.dump(rows, fh)
    return np.concatenate([res.results[r]["out"] for r in range(R)], axis=0)
